# revision 4
# baseline (speedup 1.0000x reference)
"""GQA kernel for Trainium2, sharded over 8 NeuronCores.

Problem: x[2,2048,2048] -> GQA(HQ=16 q-heads, HKV=4 kv-heads, D=128) -> out[2,2048,2048]
Sharding: core c = b*4 + h handles batch b and kv-head group h (4 q-heads).
Wq/Wk/Wv column-sharded per head group, Wo row-sharded; partial outputs
summed on host per batch.

Per-core kernel (all operands fp32):
  phase 1: qT[512,2048], kT[128,2048], vT[128,2048] projections from xT
           (feature-on-partition layout; x pre-transposed on host).
  phase 2: per q-head g, per 512-wide query block:
           scoresT[j,i] = kT_tile^T-style matmul (kT slice stationary, qT moving),
           E = exp(scores/sqrt(128)) evicted to SBUF,
           AV in out[i,d] orientation with E^T tiles stationary and
           v augmented with a ones column -> numerator + denominator in one
           PSUM accumulation; normalize, PE-transpose into attnT[f,i] layout.
  phase 3: partial outT[e,i] = WoT (stationary) @ attnT (moving), DMA to DRAM.
"""

import math

import numpy as np

B = 2
N = 2048
E = 2048
HQ = 16
G = 4
HKV = 4
D = 128
FQ = G * D  # 512 q-features per group
P = 128
NB = N // 512  # 4 moving-dim chunks
ET = E // P  # 16 contraction tiles
JT = N // P  # 16 key tiles
IB = N // 512  # 4 query blocks
SCALE = 1.0 / math.sqrt(D)

_CACHE: dict = {}


def _build_program():
    import concourse.bacc as bacc
    import concourse.tile as tile
    from concourse import mybir
    from concourse.masks import make_identity

    f32 = mybir.dt.float32
    nc = bacc.Bacc("TRN2", target_bir_lowering=False)

    xT_d = nc.dram_tensor("xT", [E, N], f32, kind="ExternalInput")
    wqT_d = nc.dram_tensor("wqT", [E, FQ], f32, kind="ExternalInput")
    wkT_d = nc.dram_tensor("wkT", [E, D], f32, kind="ExternalInput")
    wvT_d = nc.dram_tensor("wvT", [E, D], f32, kind="ExternalInput")
    woT_d = nc.dram_tensor("woT", [FQ, N], f32, kind="ExternalInput")
    outT_d = nc.dram_tensor("outT", [E, N], f32, kind="ExternalOutput")

    with tile.TileContext(nc) as tc:
        with tc.tile_pool(name="persist", bufs=1) as persist:
            ident = persist.tile([P, P], f32, tag="ident")
            make_identity(nc, ident)

            qT = [persist.tile([P, N], f32, name=f"qT{f}", tag=f"qT{f}") for f in range(G)]
            kT = persist.tile([P, N], f32, tag="kT")
            vTs = persist.tile([P, N], f32, tag="vTs")
            # v tiles [j][128, 128] + ones column at 128
            va = persist.tile([P, JT, 132], f32, tag="va")
            attnT = [persist.tile([P, N], f32, name=f"attnT{g}", tag=f"attnT{g}") for g in range(G)]
            wo_sb = persist.tile([P, G, N], f32, tag="wo_sb")

            # ---------------- phase 1: projections ----------------
            with tc.tile_pool(name="w1", bufs=1) as w1, \
                 tc.tile_pool(name="xp", bufs=4) as xp, \
                 tc.tile_pool(name="pp", bufs=1, space="PSUM") as pp, \
                 tc.tile_pool(name="pkv", bufs=2, space="PSUM") as pkv:
                wq_sb = w1.tile([P, ET, FQ], f32, tag="wq_sb")
                wk_sb = w1.tile([P, ET, D], f32, tag="wk_sb")
                wv_sb = w1.tile([P, ET, D], f32, tag="wv_sb")
                nc.sync.dma_start(
                    out=wq_sb[:], in_=wqT_d[:].rearrange("(e p) f -> p e f", p=P)
                )
                nc.sync.dma_start(
                    out=wk_sb[:], in_=wkT_d[:].rearrange("(e p) f -> p e f", p=P)
                )
                nc.sync.dma_start(
                    out=wv_sb[:], in_=wvT_d[:].rearrange("(e p) f -> p e f", p=P)
                )
                # wo is only needed in phase 3; issue its DMA now so it overlaps
                nc.sync.dma_start(
                    out=wo_sb[:], in_=woT_d[:].rearrange("(t p) n -> p t n", p=P)
                )

                for nb in range(NB):
                    qps = [pp.tile([P, 512], f32, name=f"qp{f}", tag=f"qp{f}") for f in range(G)]
                    kps = pkv.tile([P, 512], f32, tag="kp")
                    vps = pkv.tile([P, 512], f32, tag="vp")
                    for e in range(ET):
                        xt = xp.tile([P, 512], f32, tag="xt")
                        nc.sync.dma_start(
                            out=xt[:],
                            in_=xT_d[e * P:(e + 1) * P, nb * 512:(nb + 1) * 512],
                        )
                        st = e == 0
                        sp = e == ET - 1
                        for f in range(G):
                            nc.tensor.matmul(
                                qps[f][:],
                                wq_sb[:, e, f * P:(f + 1) * P],
                                xt[:],
                                start=st,
                                stop=sp,
                            )
                        nc.tensor.matmul(
                            kps[:], wk_sb[:, e, :], xt[:], start=st, stop=sp
                        )
                        nc.tensor.matmul(
                            vps[:], wv_sb[:, e, :], xt[:], start=st, stop=sp
                        )
                    for f in range(G):
                        nc.vector.tensor_copy(
                            qT[f][:, nb * 512:(nb + 1) * 512], qps[f][:]
                        )
                    nc.vector.tensor_copy(kT[:, nb * 512:(nb + 1) * 512], kps[:])
                    nc.vector.tensor_copy(vTs[:, nb * 512:(nb + 1) * 512], vps[:])

            # ---------------- v transpose + ones column ----------------
            with tc.tile_pool(name="ptr0", bufs=2, space="PSUM") as ptr0:
                for j in range(JT):
                    tp = ptr0.tile([P, P], f32, tag="tp0")
                    nc.tensor.transpose(tp[:], vTs[:, j * P:(j + 1) * P], ident[:])
                    nc.vector.tensor_copy(va[:, j, 0:P], tp[:])
                nc.vector.memset(va[:, :, P:P + 1], 1.0)

            # ---------------- phase 2: attention ----------------
            with tc.tile_pool(name="et", bufs=2) as etp, \
                 tc.tile_pool(name="small", bufs=4) as small, \
                 tc.tile_pool(name="ps", bufs=3, space="PSUM") as ps, \
                 tc.tile_pool(name="pav", bufs=3, space="PSUM") as pav, \
                 tc.tile_pool(name="ptr", bufs=2, space="PSUM") as ptr:
                for g in range(G):
                    for ib in range(IB):
                        ets = []
                        for j in range(JT):
                            sps = ps.tile([P, 512], f32, tag="sps")
                            nc.tensor.matmul(
                                sps[:],
                                kT[:, j * P:(j + 1) * P],
                                qT[g][:, ib * 512:(ib + 1) * 512],
                                start=True,
                                stop=True,
                            )
                            et = etp.tile([P, 512], f32, name=f"et{j}", tag=f"et{j}")
                            nc.scalar.activation(
                                et[:],
                                sps[:],
                                mybir.ActivationFunctionType.Exp,
                                scale=SCALE,
                            )
                            ets.append(et)
                        for isub in range(4):
                            avp = pav.tile([P, 132], f32, tag="avp")
                            for j in range(JT):
                                nc.tensor.matmul(
                                    avp[:, 0:129],
                                    ets[j][:, isub * P:(isub + 1) * P],
                                    va[:, j, 0:129],
                                    start=(j == 0),
                                    stop=(j == JT - 1),
                                )
                            rec = small.tile([P, 1], f32, tag="rec")
                            nc.vector.reciprocal(rec[:], avp[:, 128:129])
                            anorm = small.tile([P, P], f32, tag="anorm")
                            nc.vector.tensor_scalar_mul(
                                anorm[:], avp[:, 0:P], rec[:]
                            )
                            trp = ptr.tile([P, P], f32, tag="trp")
                            nc.tensor.transpose(trp[:], anorm[:], ident[:])
                            col = (ib * 4 + isub) * P
                            nc.vector.tensor_copy(
                                attnT[g][:, col:col + P], trp[:]
                            )

            # ---------------- phase 3: output projection ----------------
            with tc.tile_pool(name="po", bufs=2, space="PSUM") as po, \
                 tc.tile_pool(name="op", bufs=4) as op:
                for eo in range(ET):
                    ops_ = [po.tile([P, 512], f32, name=f"op{nb}", tag=f"op{nb}") for nb in range(NB)]
                    for f in range(G):
                        for nb in range(NB):
                            nc.tensor.matmul(
                                ops_[nb][:],
                                wo_sb[:, f, eo * P:(eo + 1) * P],
                                attnT[f][:, nb * 512:(nb + 1) * 512],
                                start=(f == 0),
                                stop=(f == G - 1),
                            )
                    for nb in range(NB):
                        ot = op.tile([P, 512], f32, tag="ot")
                        nc.vector.tensor_copy(ot[:], ops_[nb][:])
                        nc.sync.dma_start(
                            out=outT_d[eo * P:(eo + 1) * P, nb * 512:(nb + 1) * 512],
                            in_=ot[:],
                        )
    nc.finalize()
    return nc


def _get_program():
    if "nc" not in _CACHE:
        _CACHE["nc"] = _build_program()
    return _CACHE["nc"]


def _make_in_maps(x, Wq, Wk, Wv, Wo):
    xT = [np.ascontiguousarray(x[b].T) for b in range(B)]
    in_maps = []
    for c in range(8):
        b, h = c // HKV, c % HKV
        in_maps.append({
            "xT": xT[b],
            "wqT": np.ascontiguousarray(Wq[h * FQ:(h + 1) * FQ, :].T),
            "wkT": np.ascontiguousarray(Wk[h * D:(h + 1) * D, :].T),
            "wvT": np.ascontiguousarray(Wv[h * D:(h + 1) * D, :].T),
            "woT": np.ascontiguousarray(Wo[:, h * FQ:(h + 1) * FQ].T),
        })
    return in_maps


def run_spmd(in_maps, trace=False, **kw):
    from concourse.bass_utils import run_bass_kernel_spmd

    nc = _get_program()
    return run_bass_kernel_spmd(nc, in_maps, list(range(8)), trace=trace, **kw)


def kernel(x, Wq, Wk, Wv, Wo, next_token_only=0, **_ignored):
    x = np.asarray(x, dtype=np.float32)
    Wq = np.asarray(Wq, dtype=np.float32)
    Wk = np.asarray(Wk, dtype=np.float32)
    Wv = np.asarray(Wv, dtype=np.float32)
    Wo = np.asarray(Wo, dtype=np.float32)

    res = run_spmd(_make_in_maps(x, Wq, Wk, Wv, Wo))
    outs = [r["outT"] for r in res.results]
    full = np.empty((B, N, E), np.float32)
    for b in range(B):
        acc = outs[b * HKV].copy()
        for h in range(1, HKV):
            acc += outs[b * HKV + h]
        full[b] = acc.T
    return full


# revision 6
# speedup vs baseline: 2.9511x; 2.9511x over previous
"""GQA kernel for Trainium2, sharded over 8 NeuronCores.

Problem: x[2,2048,2048] -> GQA(HQ=16 q-heads, HKV=4 kv-heads, D=128) -> out[2,2048,2048]
Sharding: core c = b*4 + h handles batch b and kv-head group h (4 q-heads).
Wq/Wk/Wv column-sharded per head group, Wo row-sharded; partial outputs
summed on host per batch.

Per-core kernel (all operands fp32):
  phase 1: qT[512,2048], kT[128,2048], vT[128,2048] projections from xT
           (feature-on-partition layout; x pre-transposed on host).
  phase 2: per q-head g, per 512-wide query block:
           scoresT[j,i] = kT_tile^T-style matmul (kT slice stationary, qT moving),
           E = exp(scores/sqrt(128)) evicted to SBUF,
           AV in out[i,d] orientation with E^T tiles stationary and
           v augmented with a ones column -> numerator + denominator in one
           PSUM accumulation; normalize, PE-transpose into attnT[f,i] layout.
  phase 3: partial outT[e,i] = WoT (stationary) @ attnT (moving), DMA to DRAM.
"""

import math

import numpy as np

B = 2
N = 2048
E = 2048
HQ = 16
G = 4
HKV = 4
D = 128
FQ = G * D  # 512 q-features per group
P = 128
NB = N // 512  # 4 moving-dim chunks
ET = E // P  # 16 contraction tiles
JT = N // P  # 16 key tiles
IB = N // 512  # 4 query blocks
SCALE = 1.0 / math.sqrt(D)

_CACHE: dict = {}


def _build_program():
    import concourse.bacc as bacc
    import concourse.tile as tile
    from concourse import mybir
    from concourse.masks import make_identity

    f32 = mybir.dt.float32
    bf16 = mybir.dt.bfloat16
    nc = bacc.Bacc("TRN2", target_bir_lowering=False)

    xT_d = nc.dram_tensor("xT", [E, N], bf16, kind="ExternalInput")
    wqT_d = nc.dram_tensor("wqT", [E, FQ], bf16, kind="ExternalInput")
    wkT_d = nc.dram_tensor("wkT", [E, D], bf16, kind="ExternalInput")
    wvT_d = nc.dram_tensor("wvT", [E, D], bf16, kind="ExternalInput")
    woT_d = nc.dram_tensor("woT", [FQ, N], bf16, kind="ExternalInput")
    outT_d = nc.dram_tensor("outT", [E, N], f32, kind="ExternalOutput")

    with tile.TileContext(nc) as tc:
        with tc.tile_pool(name="persist", bufs=1) as persist:
            ident = persist.tile([P, P], bf16, tag="ident")
            make_identity(nc, ident)

            qT = [persist.tile([P, N], bf16, name=f"qT{f}", tag=f"qT{f}") for f in range(G)]
            kT = persist.tile([P, N], bf16, tag="kT")
            vTs = persist.tile([P, N], bf16, tag="vTs")
            # v tiles [j][128, 128] + ones column at 128
            va = persist.tile([P, JT, 132], bf16, tag="va")
            attnT = [persist.tile([P, N], bf16, name=f"attnT{g}", tag=f"attnT{g}") for g in range(G)]
            wo_sb = persist.tile([P, G, N], bf16, tag="wo_sb")

            # ---------------- phase 1: projections ----------------
            with tc.tile_pool(name="w1", bufs=1) as w1, \
                 tc.tile_pool(name="xp", bufs=4) as xp, \
                 tc.tile_pool(name="pp", bufs=1, space="PSUM") as pp, \
                 tc.tile_pool(name="pkv", bufs=2, space="PSUM") as pkv:
                wq_sb = w1.tile([P, ET, FQ], bf16, tag="wq_sb")
                wk_sb = w1.tile([P, ET, D], bf16, tag="wk_sb")
                wv_sb = w1.tile([P, ET, D], bf16, tag="wv_sb")
                nc.sync.dma_start(
                    out=wq_sb[:], in_=wqT_d[:].rearrange("(e p) f -> p e f", p=P)
                )
                nc.sync.dma_start(
                    out=wk_sb[:], in_=wkT_d[:].rearrange("(e p) f -> p e f", p=P)
                )
                nc.sync.dma_start(
                    out=wv_sb[:], in_=wvT_d[:].rearrange("(e p) f -> p e f", p=P)
                )
                # wo is only needed in phase 3; issue its DMA now so it overlaps
                nc.sync.dma_start(
                    out=wo_sb[:], in_=woT_d[:].rearrange("(t p) n -> p t n", p=P)
                )

                for nb in range(NB):
                    qps = [pp.tile([P, 512], f32, name=f"qp{f}", tag=f"qp{f}") for f in range(G)]
                    kps = pkv.tile([P, 512], f32, tag="kp")
                    vps = pkv.tile([P, 512], f32, tag="vp")
                    for e in range(ET):
                        xt = xp.tile([P, 512], bf16, tag="xt")
                        nc.sync.dma_start(
                            out=xt[:],
                            in_=xT_d[e * P:(e + 1) * P, nb * 512:(nb + 1) * 512],
                        )
                        st = e == 0
                        sp = e == ET - 1
                        for f in range(G):
                            nc.tensor.matmul(
                                qps[f][:],
                                wq_sb[:, e, f * P:(f + 1) * P],
                                xt[:],
                                start=st,
                                stop=sp,
                            )
                        nc.tensor.matmul(
                            kps[:], wk_sb[:, e, :], xt[:], start=st, stop=sp
                        )
                        nc.tensor.matmul(
                            vps[:], wv_sb[:, e, :], xt[:], start=st, stop=sp
                        )
                    for f in range(G):
                        nc.vector.tensor_copy(
                            qT[f][:, nb * 512:(nb + 1) * 512], qps[f][:]
                        )
                    nc.vector.tensor_copy(kT[:, nb * 512:(nb + 1) * 512], kps[:])
                    nc.vector.tensor_copy(vTs[:, nb * 512:(nb + 1) * 512], vps[:])

            # ---------------- v transpose + ones column ----------------
            with tc.tile_pool(name="ptr0", bufs=2, space="PSUM") as ptr0:
                for j in range(JT):
                    tp = ptr0.tile([P, P], bf16, tag="tp0")
                    nc.tensor.transpose(tp[:], vTs[:, j * P:(j + 1) * P], ident[:])
                    nc.vector.tensor_copy(va[:, j, 0:P], tp[:])
                nc.vector.memset(va[:, :, P:P + 1], 1.0)

            # ---------------- phase 2: attention ----------------
            with tc.tile_pool(name="et", bufs=2) as etp, \
                 tc.tile_pool(name="small", bufs=4) as small, \
                 tc.tile_pool(name="ps", bufs=3, space="PSUM") as ps, \
                 tc.tile_pool(name="pav", bufs=3, space="PSUM") as pav, \
                 tc.tile_pool(name="ptr", bufs=2, space="PSUM") as ptr:
                for g in range(G):
                    for ib in range(IB):
                        ets = []
                        for j in range(JT):
                            sps = ps.tile([P, 512], f32, tag="sps")
                            nc.tensor.matmul(
                                sps[:],
                                kT[:, j * P:(j + 1) * P],
                                qT[g][:, ib * 512:(ib + 1) * 512],
                                start=True,
                                stop=True,
                            )
                            et = etp.tile([P, 512], bf16, name=f"et{j}", tag=f"et{j}")
                            nc.scalar.activation(
                                et[:],
                                sps[:],
                                mybir.ActivationFunctionType.Exp,
                                scale=SCALE,
                            )
                            ets.append(et)
                        for isub in range(4):
                            avp = pav.tile([P, 132], f32, tag="avp")
                            for j in range(JT):
                                nc.tensor.matmul(
                                    avp[:, 0:129],
                                    ets[j][:, isub * P:(isub + 1) * P],
                                    va[:, j, 0:129],
                                    start=(j == 0),
                                    stop=(j == JT - 1),
                                )
                            rec = small.tile([P, 1], f32, tag="rec")
                            nc.vector.reciprocal(rec[:], avp[:, 128:129])
                            anorm = small.tile([P, P], bf16, tag="anorm")
                            nc.vector.tensor_scalar_mul(
                                anorm[:], avp[:, 0:P], rec[:]
                            )
                            trp = ptr.tile([P, P], bf16, tag="trp")
                            nc.tensor.transpose(trp[:], anorm[:], ident[:])
                            col = (ib * 4 + isub) * P
                            nc.vector.tensor_copy(
                                attnT[g][:, col:col + P], trp[:]
                            )

            # ---------------- phase 3: output projection ----------------
            with tc.tile_pool(name="po", bufs=2, space="PSUM") as po, \
                 tc.tile_pool(name="op", bufs=4) as op:
                for eo in range(ET):
                    ops_ = [po.tile([P, 512], f32, name=f"op{nb}", tag=f"op{nb}") for nb in range(NB)]
                    for f in range(G):
                        for nb in range(NB):
                            nc.tensor.matmul(
                                ops_[nb][:],
                                wo_sb[:, f, eo * P:(eo + 1) * P],
                                attnT[f][:, nb * 512:(nb + 1) * 512],
                                start=(f == 0),
                                stop=(f == G - 1),
                            )
                    for nb in range(NB):
                        ot = op.tile([P, 512], f32, tag="ot")
                        nc.vector.tensor_copy(ot[:], ops_[nb][:])
                        nc.sync.dma_start(
                            out=outT_d[eo * P:(eo + 1) * P, nb * 512:(nb + 1) * 512],
                            in_=ot[:],
                        )
    nc.finalize()
    return nc


def _get_program():
    if "nc" not in _CACHE:
        _CACHE["nc"] = _build_program()
    return _CACHE["nc"]


def _make_in_maps(x, Wq, Wk, Wv, Wo):
    import ml_dtypes

    bf = ml_dtypes.bfloat16
    xT = [np.ascontiguousarray(x[b].T).astype(bf) for b in range(B)]
    in_maps = []
    for c in range(8):
        b, h = c // HKV, c % HKV
        in_maps.append({
            "xT": xT[b],
            "wqT": np.ascontiguousarray(Wq[h * FQ:(h + 1) * FQ, :].T).astype(bf),
            "wkT": np.ascontiguousarray(Wk[h * D:(h + 1) * D, :].T).astype(bf),
            "wvT": np.ascontiguousarray(Wv[h * D:(h + 1) * D, :].T).astype(bf),
            "woT": np.ascontiguousarray(Wo[:, h * FQ:(h + 1) * FQ].T).astype(bf),
        })
    return in_maps


def run_spmd(in_maps, trace=False, **kw):
    from concourse.bass_utils import run_bass_kernel_spmd

    nc = _get_program()
    return run_bass_kernel_spmd(nc, in_maps, list(range(8)), trace=trace, **kw)


def kernel(x, Wq, Wk, Wv, Wo, next_token_only=0, **_ignored):
    x = np.asarray(x, dtype=np.float32)
    Wq = np.asarray(Wq, dtype=np.float32)
    Wk = np.asarray(Wk, dtype=np.float32)
    Wv = np.asarray(Wv, dtype=np.float32)
    Wo = np.asarray(Wo, dtype=np.float32)

    res = run_spmd(_make_in_maps(x, Wq, Wk, Wv, Wo))
    outs = [r["outT"] for r in res.results]
    full = np.empty((B, N, E), np.float32)
    for b in range(B):
        acc = outs[b * HKV].copy()
        for h in range(1, HKV):
            acc += outs[b * HKV + h]
        full[b] = acc.T
    return full
